# revision 5
# baseline (speedup 1.0000x reference)
"""Char-LSTM kernel for Trainium2 (8 NeuronCores, data parallel).

Strategy (v2 — activation-bound design)
---------------------------------------
Host side:
  * Sort words by length, deal them into per-core blocks of 512 words of a
    single length each (padded with dummies); leftovers fold into the
    length-16 block which runs per-step h capture.
  * Per (block, t) ship an x-slab [33, 512] bf16 = [embedded chars; ones row]
    transposed, so the whole gate pre-activation is ONE matmul per bank:
    rhs = [h | ones | x] (K = 97), lhsT = [W_hh.T | b | W_ih.T].

Device side per group-step (two 512-word blocks A/B sharing 128 partitions):
  * 8 matmuls (4 gate banks x 2 halves, M=64, K=97, N=512) into one
    [128, 2048] PSUM tile laid out as banks [i | f | o | g].
  * ONE sigmoid over all 4 banks [128, 2048] -> bf16. The g-bank weights are
    pre-scaled by 2 so tanh(g) = 2*sigmoid(2g) - 1; the cell state is kept
    halved (c' = c/2) which makes the update exact:
        t1 = (sg - 0.5) * si            (= i*g / 2, fused DVE op)
        c' = sf * c' + t1
        tc = tanh(2 * c')               (ONE act instr, scale=2)
        h  = so * tc                    (bf16, written straight into the
                                         next step's matmul rhs tile)
  * h lives in rows 0:64 of the A-slab (rhs base 0) and rows 64:128 of the
    B-slab (rhs base 31), so no partition shifts are ever needed.
  Groups are emitted interleaved ~3 wide so independent recurrence chains
  pipeline across the Tensor/Scalar/Vector engines; the Scalar (activation)
  engine is the roofline at ~2.5us per group-step.
"""

import os
import sys

for _p in ("/opt/trn_rl_repo", "/root/.axon_site/_ro/trn_rl_repo"):
    if os.path.isdir(_p) and _p not in sys.path:
        sys.path.insert(0, _p)

import numpy as np
import ml_dtypes

BF16 = ml_dtypes.bfloat16

H = 64          # hidden size
E = 32          # char embedding size
V = 100         # vocab
MAXL = 16       # max word length
BLK = 512       # words per block (one half of a group)
NCORES = 8
GATE4 = 4 * H   # 256
KDIM = H + 1 + E  # 97: [h | ones | x]

# torch gate order in the weights is [i, f, g, o]; we stage banks as
# [i, f, o, g] so one sigmoid covers the whole [128, 2048] span (g uses the
# 2*sigmoid(2x)-1 tanh identity).
_GATE_PERM = np.concatenate([
    np.arange(0, 64),        # i
    np.arange(64, 128),      # f
    np.arange(192, 256),     # o
    np.arange(128, 192),     # g
])

INTERLEAVE = int(os.environ.get("LSTM_INTERLEAVE", "3"))
SKIPB = os.environ.get("LSTM_SKIPB", "1") == "1"
_PROGRAM_CACHE = {}


# --------------------------------------------------------------------------
# Host-side planning
# --------------------------------------------------------------------------

def _plan(lengths):
    """Assign words to (core, block, column) slots.

    Returns dict with:
      blocks: list (shared across cores) of dicts {L, is_ov, ov_idx}
      groups: list of dicts {a, b, steps} (block indices)
      sched:  emission order list of (group_idx, t)
      assign: per core: list of np arrays [BLK] of word ids (-1 = dummy),
              aligned with blocks
    """
    n = lengths.shape[0]
    lengths = lengths.astype(np.int64)
    order = np.argsort(lengths, kind="stable")

    per_core_words = [[] for _ in range(NCORES)]   # per core: list of [BLK] arrays
    block_meta = []                                # shared: (L, is_ov)

    leftovers = []
    for L in range(1, MAXL + 1):
        idx = order[np.searchsorted(lengths, L, side="left", sorter=order):
                    np.searchsorted(lengths, L, side="right", sorter=order)]
        take = idx[: NCORES * BLK]
        leftovers.append(idx[NCORES * BLK:])
        arr = np.full(NCORES * BLK, -1, dtype=np.int64)
        arr[: take.shape[0]] = take
        arr = arr.reshape(NCORES, BLK)
        for c in range(NCORES):
            per_core_words[c].append(arr[c])
        block_meta.append((L, False))

    leftovers = np.concatenate(leftovers) if leftovers else np.empty(0, np.int64)

    # Try to fold leftover words into the free slots of the length-16 block
    # (which then runs per-step capture); fall back to dedicated overflow
    # blocks when they don't fit.
    l16 = MAXL - 1  # index of the length-16 block in block_meta order
    free16 = [int((per_core_words[c][l16] < 0).sum()) for c in range(NCORES)]
    if leftovers.shape[0] <= sum(free16):
        block_meta[l16] = (MAXL, True)
        pos = 0
        for c in range(NCORES):
            k = min(free16[c], leftovers.shape[0] - pos)
            if k > 0:
                arr = per_core_words[c][l16]
                slots = np.nonzero(arr < 0)[0][:k]
                arr[slots] = leftovers[pos:pos + k]
                pos += k
        leftovers = leftovers[:0]

    if leftovers.shape[0]:
        n_ov = -(-leftovers.shape[0] // (NCORES * BLK))
        ov = np.full(n_ov * NCORES * BLK, -1, dtype=np.int64)
        ov[: leftovers.shape[0]] = leftovers
        ov = ov.reshape(n_ov, NCORES, BLK)
        for i in range(n_ov):
            for c in range(NCORES):
                per_core_words[c].append(ov[i, c])
            block_meta.append((MAXL, True))

    if len(block_meta) % 2 == 1:
        for c in range(NCORES):
            per_core_words[c].append(np.full(BLK, -1, dtype=np.int64))
        block_meta.append((1, False))

    # Sort blocks: descending length, overflow blocks first among equals so
    # they pair with the longest regular block.
    nb = len(block_meta)
    key = sorted(range(nb), key=lambda b: (-block_meta[b][0], not block_meta[b][1]))
    blocks = []
    ov_count = 0
    for b in key:
        L, is_ov = block_meta[b]
        blocks.append({"L": L, "is_ov": is_ov,
                       "ov_idx": (ov_count if is_ov else -1), "orig": b})
        if is_ov:
            ov_count += 1

    assign = [[per_core_words[c][blocks[i]["orig"]] for i in range(nb)]
              for c in range(NCORES)]

    groups = []
    for i in range(0, nb, 2):
        groups.append({"a": i, "b": i + 1,
                       "steps": max(blocks[i]["L"], blocks[i + 1]["L"])})

    # Fixed-active-set interleave: at most INTERLEAVE groups are ever live
    # (bounds tile-pool pressure; more live groups deadlock the Tile
    # scheduler on the long-lived c/slab tiles). Round-robin one step each;
    # when a group finishes, admit the next-longest from the queue.
    remaining = [g["steps"] for g in groups]
    next_t = [0] * len(groups)
    queue = sorted(range(len(groups)), key=lambda g: -remaining[g])
    active = queue[:INTERLEAVE]
    queue = queue[INTERLEAVE:]
    sched = []
    while active:
        for g in list(active):
            sched.append((g, next_t[g]))
            next_t[g] += 1
            remaining[g] -= 1
            if remaining[g] == 0:
                i = active.index(g)
                if queue:
                    active[i] = queue.pop(0)
                else:
                    active.pop(i)

    # capture steps: for each capture block, the union (over cores) of
    # final steps of its words with length < MAXL, plus MAXL-1 (so length-16
    # words folded into a capture block are also covered).
    for bi, blk in enumerate(blocks):
        if not blk["is_ov"]:
            continue
        steps = set()
        for c in range(NCORES):
            w = assign[c][bi]
            w = w[w >= 0]
            steps.update((lengths[w] - 1).tolist())
        blk["cap_steps"] = tuple(sorted(steps))

    return {"blocks": blocks, "groups": groups, "sched": sched,
            "assign": assign, "n_ov": ov_count}


def _build_xslabs(plan, chars, emb16):
    """Per-core x-slab tensors [n_blocks*MAXL, 64, BLK] bf16, indexed by
    block_idx*MAXL + t. The 64-row slab is DMA'd whole so the matmul K range
    is a full, partition-aligned 128 rows (zero rows x zero weights = 0).

    A-half blocks (even index), DMA'd to partitions 64:128:
      row 0 = ones, rows 1:33 = emb[ch].T, rows 33:64 = zeros.
    B-half blocks (odd index), DMA'd to partitions 0:64:
      rows 0:32 = emb[ch].T, row 32 = ones, rows 33:64 = zeros.
    """
    blocks = plan["blocks"]
    nb = len(blocks)
    out = []
    for c in range(NCORES):
        xs = np.zeros((nb * MAXL, 64, BLK), dtype=BF16)
        for bi, blk in enumerate(blocks):
            words = plan["assign"][c][bi]
            w = np.where(words < 0, 0, words)
            L = blk["L"]
            ch = chars[w, :L]                       # [BLK, L]
            xt = emb16[ch]                          # [BLK, L, E]
            xt = np.ascontiguousarray(np.transpose(xt, (1, 2, 0)))  # [L, E, BLK]
            base = bi * MAXL
            if bi % 2 == 0:
                xs[base:base + L, 1:33, :] = xt
                xs[base:base + L, 0, :] = 1.0
            else:
                xs[base:base + L, 0:32, :] = xt
                xs[base:base + L, 32, :] = 1.0
        out.append(xs)
    return out


# --------------------------------------------------------------------------
# Device program
# --------------------------------------------------------------------------

def _build_program(plan_sig, blocks, groups, sched, n_ov, variant="full",
                   reps=1):
    import concourse.bass as bass
    import concourse.tile as tile
    from concourse import bacc, mybir
    from contextlib import nullcontext

    do_mm = variant not in ("nomm",)
    do_act = variant not in ("noact",)
    do_dma = variant not in ("nodma",)

    f32 = mybir.dt.float32
    bf16 = mybir.dt.bfloat16
    Sigmoid = mybir.ActivationFunctionType.Sigmoid
    Tanh = mybir.ActivationFunctionType.Tanh
    ADD = mybir.AluOpType.add
    MULT = mybir.AluOpType.mult

    n_blocks = len(blocks)
    n_slabs = n_blocks * MAXL

    nc = bacc.Bacc("TRN2", target_bir_lowering=False, debug=False,
                   num_devices=NCORES)
    xsl_d = nc.dram_tensor("xsl", [n_slabs, 64, BLK], bf16,
                           kind="ExternalInput")
    ga_d = nc.dram_tensor("ga", [128, GATE4], bf16, kind="ExternalInput")
    gb_d = nc.dram_tensor("gb", [128, GATE4], bf16, kind="ExternalInput")
    out_d = nc.dram_tensor("out", [n_blocks, H, BLK], bf16,
                           kind="ExternalOutput")
    ov_d = nc.dram_tensor("ov", [max(1, n_ov) * MAXL, H, BLK], bf16,
                          kind="ExternalOutput")

    with tile.TileContext(nc) as tc:
        with (
            tc.tile_pool(name="consts", bufs=1) as consts,
            tc.tile_pool(name="slabs", bufs=14) as slabs,
            tc.tile_pool(name="psum", bufs=2, space="PSUM") as psump,
            tc.tile_pool(name="sig", bufs=3) as sigp,
            tc.tile_pool(name="t1_", bufs=4) as t1p,
            tc.tile_pool(name="t2_", bufs=4) as t2p,
            tc.tile_pool(name="tc_", bufs=3) as tcp,
            tc.tile_pool(name="state", bufs=5) as statep,
        ):
            ga = consts.tile([128, GATE4], bf16, tag="ga")
            gb = consts.tile([128, GATE4], bf16, tag="gb")
            nc.sync.dma_start(out=ga[:], in_=ga_d[:])
            nc.sync.dma_start(out=gb[:], in_=gb_d[:])

            loop_cm = tc.For_i(0, reps, 1) if reps > 1 else nullcontext()
            with loop_cm:
                gstate = {}
                for (g, t) in sched:
                    grp = groups[g]
                    a, b = blocks[grp["a"]], blocks[grp["b"]]
                    La, Lb = a["L"], b["L"]
                    first = (t == 0)
                    b_act = (t < Lb) or not SKIPB
                    sl = slice(0, 128 if b_act else 64)

                    if first:
                        sa = slabs.tile([128, BLK], bf16, tag="slab", name="sa0")
                        sb = slabs.tile([128, BLK], bf16, tag="slab", name="sb0")
                        # zero h rows: gates_0 = W_ih x + b exactly
                        nc.gpsimd.memset(sa[0:64, :], 0.0)
                        nc.gpsimd.memset(sb[64:128, :], 0.0)
                        if do_dma:
                            nc.sync.dma_start(out=sa[64:128, :],
                                              in_=xsl_d[grp["a"] * MAXL])
                            nc.sync.dma_start(out=sb[0:64, :],
                                              in_=xsl_d[grp["b"] * MAXL])
                        st = gstate[g] = {
                            "sa": sa, "sb": sb,
                            "c": statep.tile([128, BLK], f32, tag="c", name="c"),
                        }
                    else:
                        st = gstate[g]

                    ps = psump.tile([128, 4 * BLK], f32, tag="ps")
                    if do_mm:
                        for q in range(4):
                            qs = slice(64 * q, 64 * q + 64)
                            cs = slice(BLK * q, BLK * q + BLK)
                            nc.tensor.matmul(ps[0:64, cs], ga[:, qs],
                                             st["sa"][:, :],
                                             start=True, stop=True,
                                             tile_position=(0, 0))
                            if b_act:
                                nc.tensor.matmul(ps[64:128, cs], gb[:, qs],
                                                 st["sb"][:, :],
                                                 start=True, stop=True,
                                                 tile_position=(0, 64))

                    # next-step rhs tiles (also the h target / output source)
                    na = slabs.tile([128, BLK], bf16, tag="slab", name="na")
                    if do_dma and t + 1 < La:
                        nc.sync.dma_start(out=na[64:128, :],
                                          in_=xsl_d[grp["a"] * MAXL + t + 1])
                    if b_act:
                        nb_ = slabs.tile([128, BLK], bf16, tag="slab", name="nb")
                        if do_dma and t + 1 < Lb:
                            nc.sync.dma_start(out=nb_[0:64, :],
                                              in_=xsl_d[grp["b"] * MAXL + t + 1])

                    if do_act:
                        sig = sigp.tile([128, 4 * BLK], bf16, tag="sig")
                        nc.scalar.activation(out=sig[sl, :], in_=ps[sl, :],
                                             func=Sigmoid)
                        # t1 = (sg - 0.5) * si  (= i*g / 2)
                        tgt = st["c"] if first else \
                            t1p.tile([128, BLK], bf16, tag="t1")
                        nc.vector.scalar_tensor_tensor(
                            tgt[sl, :], sig[sl, 3 * BLK:4 * BLK], -0.5,
                            sig[sl, 0:BLK], ADD, MULT)
                        if not first:
                            t2 = t2p.tile([128, BLK], f32, tag="t2")
                            nc.vector.tensor_mul(t2[sl, :],
                                                 sig[sl, BLK:2 * BLK],
                                                 st["c"][sl, :])
                            nc.vector.tensor_add(st["c"][sl, :], tgt[sl, :],
                                                 t2[sl, :])
                        tch = tcp.tile([128, BLK], bf16, tag="tc")
                        nc.scalar.activation(out=tch[sl, :], in_=st["c"][sl, :],
                                             func=Tanh, scale=2.0)
                        nc.vector.tensor_mul(na[0:64, :],
                                             sig[0:64, 2 * BLK:3 * BLK],
                                             tch[0:64, :])
                        if b_act:
                            nc.vector.tensor_mul(nb_[64:128, :],
                                                 sig[64:128, 2 * BLK:3 * BLK],
                                                 tch[64:128, :])

                    if do_act and do_dma:
                        if t == La - 1:
                            nc.sync.dma_start(out=out_d[grp["a"]],
                                              in_=na[0:64, :])
                        if b_act and t == Lb - 1:
                            nc.sync.dma_start(out=out_d[grp["b"]],
                                              in_=nb_[64:128, :])
                        if a["is_ov"] and t in a.get("cap_steps", ()):
                            nc.sync.dma_start(
                                out=ov_d[a["ov_idx"] * MAXL + t],
                                in_=na[0:64, :])
                        if b_act and b["is_ov"] and t in b.get("cap_steps", ()):
                            nc.sync.dma_start(
                                out=ov_d[b["ov_idx"] * MAXL + t],
                                in_=nb_[64:128, :])

                    st["sa"] = na
                    if b_act:
                        st["sb"] = nb_

    nc.compile()
    return nc


# --------------------------------------------------------------------------
# Entry point
# --------------------------------------------------------------------------

def kernel(emb, W_ih, W_hh, b_ih, b_hh, chars, lengths):
    from concourse.bass_utils import run_bass_kernel_spmd

    emb = np.asarray(emb, dtype=np.float32)
    W_ih = np.asarray(W_ih, dtype=np.float32)
    W_hh = np.asarray(W_hh, dtype=np.float32)
    b_ih = np.asarray(b_ih, dtype=np.float32)
    b_hh = np.asarray(b_hh, dtype=np.float32)
    chars = np.asarray(chars)
    lengths_np = np.asarray(lengths)

    n = chars.shape[0]

    # --- weight prep -------------------------------------------------------
    # Banks ordered [i, f, o, g]; g-bank scaled by 2 (tanh via sigmoid).
    scale = np.ones((1, GATE4), dtype=np.float32)
    scale[0, 3 * H:] = 2.0
    WihT = W_ih[_GATE_PERM].T * scale               # [E, 4H]
    WhhT = W_hh[_GATE_PERM].T * scale               # [H, 4H]
    bias = ((b_ih + b_hh)[_GATE_PERM] * scale[0])[None, :]  # [1, 4H]
    gA = np.zeros((128, GATE4), dtype=BF16)
    gA[0:H] = WhhT.astype(BF16)
    gA[H:H + 1] = bias.astype(BF16)
    gA[H + 1:H + 1 + E] = WihT.astype(BF16)
    gB = np.zeros((128, GATE4), dtype=BF16)
    gB[0:E] = WihT.astype(BF16)
    gB[E:E + 1] = bias.astype(BF16)
    gB[64:128] = WhhT.astype(BF16)

    # --- word assignment ---------------------------------------------------
    plan = _plan(lengths_np)
    blocks, groups, sched = plan["blocks"], plan["groups"], plan["sched"]

    sig = (tuple((b["L"], b["is_ov"], b.get("cap_steps", ())) for b in blocks),
           tuple(sched))
    key = hash(sig)
    if key not in _PROGRAM_CACHE:
        _PROGRAM_CACHE[key] = _build_program(sig, blocks, groups, sched,
                                             plan["n_ov"])
    nc = _PROGRAM_CACHE[key]

    emb16 = emb.astype(BF16)
    xsls = _build_xslabs(plan, chars, emb16)
    in_maps = [{"xsl": xsls[c], "ga": gA, "gb": gB} for c in range(NCORES)]

    res = run_bass_kernel_spmd(nc, in_maps, core_ids=list(range(NCORES)))
    kernel._last_nc = nc
    kernel._last_in_maps = in_maps

    # --- gather results ----------------------------------------------------
    outs = np.stack([np.asarray(r["out"], dtype=np.float32)
                     for r in res.results])          # [8, nb, H, BLK]
    ovs = np.stack([np.asarray(r["ov"], dtype=np.float32)
                    for r in res.results])           # [8, n_ov*16, H, BLK]

    result = np.empty((n, H), dtype=np.float32)
    for c in range(NCORES):
        for bi, blk in enumerate(blocks):
            words = plan["assign"][c][bi]
            valid = words >= 0
            if not valid.any():
                continue
            w = words[valid]
            cols = np.nonzero(valid)[0]
            if blk["is_ov"]:
                steps = lengths_np[w].astype(np.int64) - 1
                result[w] = ovs[c, blk["ov_idx"] * MAXL + steps, :, cols]
            else:
                result[w] = outs[c, bi, :, cols]
    return result


# revision 7
# speedup vs baseline: 1.2279x; 1.2279x over previous
"""Char-LSTM kernel for Trainium2 (8 NeuronCores, data parallel).

Strategy (v2 — activation-bound design)
---------------------------------------
Host side:
  * Sort words by length, deal them into per-core blocks of 512 words of a
    single length each (padded with dummies); leftovers fold into the
    length-16 block which runs per-step h capture.
  * Per (block, t) ship an x-slab [33, 512] bf16 = [embedded chars; ones row]
    transposed, so the whole gate pre-activation is ONE matmul per bank:
    rhs = [h | ones | x] (K = 97), lhsT = [W_hh.T | b | W_ih.T].

Device side per group-step (two 512-word blocks A/B sharing 128 partitions):
  * 8 matmuls (4 gate banks x 2 halves, M=64, K=97, N=512) into one
    [128, 2048] PSUM tile laid out as banks [i | f | o | g].
  * ONE sigmoid over all 4 banks [128, 2048] -> bf16. The g-bank weights are
    pre-scaled by 2 so tanh(g) = 2*sigmoid(2g) - 1; the cell state is kept
    halved (c' = c/2) which makes the update exact:
        t1 = (sg - 0.5) * si            (= i*g / 2, fused DVE op)
        c' = sf * c' + t1
        tc = tanh(2 * c')               (ONE act instr, scale=2)
        h  = so * tc                    (bf16, written straight into the
                                         next step's matmul rhs tile)
  * h lives in rows 0:64 of the A-slab (rhs base 0) and rows 64:128 of the
    B-slab (rhs base 31), so no partition shifts are ever needed.
  Groups are emitted interleaved ~3 wide so independent recurrence chains
  pipeline across the Tensor/Scalar/Vector engines; the Scalar (activation)
  engine is the roofline at ~2.5us per group-step.
"""

import os
import sys

for _p in ("/opt/trn_rl_repo", "/root/.axon_site/_ro/trn_rl_repo"):
    if os.path.isdir(_p) and _p not in sys.path:
        sys.path.insert(0, _p)

import numpy as np
import ml_dtypes

BF16 = ml_dtypes.bfloat16

H = 64          # hidden size
E = 32          # char embedding size
V = 100         # vocab
MAXL = 16       # max word length
BLK = 512       # words per block (one half of a group)
NCORES = 8
GATE4 = 4 * H   # 256
KDIM = H + 1 + E  # 97: [h | ones | x]

# torch gate order in the weights is [i, f, g, o]; we stage banks as
# [i, f, o, g] so one sigmoid covers the whole [128, 2048] span (g uses the
# 2*sigmoid(2x)-1 tanh identity).
_GATE_PERM = np.concatenate([
    np.arange(0, 64),        # i
    np.arange(64, 128),      # f
    np.arange(192, 256),     # o
    np.arange(128, 192),     # g
])

INTERLEAVE = int(os.environ.get("LSTM_INTERLEAVE", "4"))
SKIPB = os.environ.get("LSTM_SKIPB", "1") == "1"
_PROGRAM_CACHE = {}


# --------------------------------------------------------------------------
# Host-side planning
# --------------------------------------------------------------------------

def _plan(lengths):
    """Assign words to (core, block, column) slots.

    Returns dict with:
      blocks: list (shared across cores) of dicts {L, is_ov, ov_idx}
      groups: list of dicts {a, b, steps} (block indices)
      sched:  emission order list of (group_idx, t)
      assign: per core: list of np arrays [BLK] of word ids (-1 = dummy),
              aligned with blocks
    """
    n = lengths.shape[0]
    lengths = lengths.astype(np.int64)
    order = np.argsort(lengths, kind="stable")

    per_core_words = [[] for _ in range(NCORES)]   # per core: list of [BLK] arrays
    block_meta = []                                # shared: (L, is_ov)

    leftovers = []
    for L in range(1, MAXL + 1):
        idx = order[np.searchsorted(lengths, L, side="left", sorter=order):
                    np.searchsorted(lengths, L, side="right", sorter=order)]
        take = idx[: NCORES * BLK]
        leftovers.append(idx[NCORES * BLK:])
        arr = np.full(NCORES * BLK, -1, dtype=np.int64)
        arr[: take.shape[0]] = take
        arr = arr.reshape(NCORES, BLK)
        for c in range(NCORES):
            per_core_words[c].append(arr[c])
        block_meta.append((L, False))

    leftovers = np.concatenate(leftovers) if leftovers else np.empty(0, np.int64)

    # Try to fold leftover words into the free slots of the length-16 block
    # (which then runs per-step capture); fall back to dedicated overflow
    # blocks when they don't fit.
    l16 = MAXL - 1  # index of the length-16 block in block_meta order
    free16 = [int((per_core_words[c][l16] < 0).sum()) for c in range(NCORES)]
    if leftovers.shape[0] <= sum(free16):
        block_meta[l16] = (MAXL, True)
        pos = 0
        for c in range(NCORES):
            k = min(free16[c], leftovers.shape[0] - pos)
            if k > 0:
                arr = per_core_words[c][l16]
                slots = np.nonzero(arr < 0)[0][:k]
                arr[slots] = leftovers[pos:pos + k]
                pos += k
        leftovers = leftovers[:0]

    if leftovers.shape[0]:
        n_ov = -(-leftovers.shape[0] // (NCORES * BLK))
        ov = np.full(n_ov * NCORES * BLK, -1, dtype=np.int64)
        ov[: leftovers.shape[0]] = leftovers
        ov = ov.reshape(n_ov, NCORES, BLK)
        for i in range(n_ov):
            for c in range(NCORES):
                per_core_words[c].append(ov[i, c])
            block_meta.append((MAXL, True))

    if len(block_meta) % 2 == 1:
        for c in range(NCORES):
            per_core_words[c].append(np.full(BLK, -1, dtype=np.int64))
        block_meta.append((1, False))

    # Sort blocks: descending length, overflow blocks first among equals so
    # they pair with the longest regular block.
    nb = len(block_meta)
    key = sorted(range(nb), key=lambda b: (-block_meta[b][0], not block_meta[b][1]))
    blocks = []
    ov_count = 0
    for b in key:
        L, is_ov = block_meta[b]
        blocks.append({"L": L, "is_ov": is_ov,
                       "ov_idx": (ov_count if is_ov else -1), "orig": b})
        if is_ov:
            ov_count += 1

    assign = [[per_core_words[c][blocks[i]["orig"]] for i in range(nb)]
              for c in range(NCORES)]

    groups = []
    for i in range(0, nb, 2):
        groups.append({"a": i, "b": i + 1,
                       "steps": max(blocks[i]["L"], blocks[i + 1]["L"])})

    # Fixed-active-set interleave: at most INTERLEAVE groups are ever live
    # (bounds tile-pool pressure; more live groups deadlock the Tile
    # scheduler on the long-lived c/slab tiles). Round-robin one step each;
    # when a group finishes, admit the next-longest from the queue.
    remaining = [g["steps"] for g in groups]
    next_t = [0] * len(groups)
    queue = sorted(range(len(groups)), key=lambda g: -remaining[g])
    active = queue[:INTERLEAVE]
    queue = queue[INTERLEAVE:]
    sched = []
    while active:
        for g in list(active):
            sched.append((g, next_t[g]))
            next_t[g] += 1
            remaining[g] -= 1
            if remaining[g] == 0:
                i = active.index(g)
                if queue:
                    active[i] = queue.pop(0)
                else:
                    active.pop(i)

    # capture steps: for each capture block, the union (over cores) of
    # final steps of its words with length < MAXL, plus MAXL-1 (so length-16
    # words folded into a capture block are also covered).
    for bi, blk in enumerate(blocks):
        if not blk["is_ov"]:
            continue
        steps = set()
        for c in range(NCORES):
            w = assign[c][bi]
            w = w[w >= 0]
            steps.update((lengths[w] - 1).tolist())
        blk["cap_steps"] = tuple(sorted(steps))

    return {"blocks": blocks, "groups": groups, "sched": sched,
            "assign": assign, "n_ov": ov_count}


def _build_xslabs(plan, chars, emb16):
    """Per-core x-slab tensors [n_blocks*MAXL, 64, BLK] bf16, indexed by
    block_idx*MAXL + t. The 64-row slab is DMA'd whole so the matmul K range
    is a full, partition-aligned 128 rows (zero rows x zero weights = 0).

    A-half blocks (even index), DMA'd to partitions 64:128:
      row 0 = ones, rows 1:33 = emb[ch].T, rows 33:64 = zeros.
    B-half blocks (odd index), DMA'd to partitions 0:64:
      rows 0:32 = emb[ch].T, row 32 = ones, rows 33:64 = zeros.
    """
    blocks = plan["blocks"]
    nb = len(blocks)
    out = []
    for c in range(NCORES):
        xs = np.zeros((nb * MAXL, 64, BLK), dtype=BF16)
        for bi, blk in enumerate(blocks):
            words = plan["assign"][c][bi]
            w = np.where(words < 0, 0, words)
            L = blk["L"]
            ch = chars[w, :L]                       # [BLK, L]
            xt = emb16[ch]                          # [BLK, L, E]
            xt = np.ascontiguousarray(np.transpose(xt, (1, 2, 0)))  # [L, E, BLK]
            base = bi * MAXL
            if bi % 2 == 0:
                xs[base:base + L, 1:33, :] = xt
                xs[base:base + L, 0, :] = 1.0
            else:
                xs[base:base + L, 0:32, :] = xt
                xs[base:base + L, 32, :] = 1.0
        out.append(xs)
    return out


# --------------------------------------------------------------------------
# Device program
# --------------------------------------------------------------------------

def _build_program(plan_sig, blocks, groups, sched, n_ov, variant="full",
                   reps=1):
    import concourse.bass as bass
    import concourse.tile as tile
    from concourse import bacc, mybir
    from contextlib import nullcontext

    do_mm = variant not in ("nomm",)
    do_act = variant not in ("noact",)
    do_dma = variant not in ("nodma",)

    f32 = mybir.dt.float32
    bf16 = mybir.dt.bfloat16
    Sigmoid = mybir.ActivationFunctionType.Sigmoid
    Tanh = mybir.ActivationFunctionType.Tanh
    ADD = mybir.AluOpType.add
    MULT = mybir.AluOpType.mult

    n_blocks = len(blocks)
    n_slabs = n_blocks * MAXL

    nc = bacc.Bacc("TRN2", target_bir_lowering=False, debug=False,
                   num_devices=NCORES)
    xsl_d = nc.dram_tensor("xsl", [n_slabs, 64, BLK], bf16,
                           kind="ExternalInput")
    ga_d = nc.dram_tensor("ga", [128, GATE4], bf16, kind="ExternalInput")
    gb_d = nc.dram_tensor("gb", [128, GATE4], bf16, kind="ExternalInput")
    out_d = nc.dram_tensor("out", [n_blocks, H, BLK], bf16,
                           kind="ExternalOutput")
    ov_d = nc.dram_tensor("ov", [max(1, n_ov) * MAXL, H, BLK], bf16,
                          kind="ExternalOutput")

    with tile.TileContext(nc) as tc:
        with (
            tc.tile_pool(name="consts", bufs=1) as consts,
            tc.tile_pool(name="slabs", bufs=18) as slabs,
            tc.tile_pool(name="psum", bufs=2, space="PSUM") as psump,
            tc.tile_pool(name="sig", bufs=4) as sigp,
            tc.tile_pool(name="t1_", bufs=4) as t1p,
            tc.tile_pool(name="t2_", bufs=4) as t2p,
            tc.tile_pool(name="tc_", bufs=4) as tcp,
            tc.tile_pool(name="state", bufs=6) as statep,
        ):
            ga = consts.tile([128, GATE4], bf16, tag="ga")
            gb = consts.tile([128, GATE4], bf16, tag="gb")
            nc.sync.dma_start(out=ga[:], in_=ga_d[:])
            nc.sync.dma_start(out=gb[:], in_=gb_d[:])

            loop_cm = tc.For_i(0, reps, 1) if reps > 1 else nullcontext()
            with loop_cm:
                gstate = {}
                for (g, t) in sched:
                    grp = groups[g]
                    a, b = blocks[grp["a"]], blocks[grp["b"]]
                    La, Lb = a["L"], b["L"]
                    first = (t == 0)
                    b_act = (t < Lb) or not SKIPB
                    sl = slice(0, 128 if b_act else 64)

                    if first:
                        sa = slabs.tile([128, BLK], bf16, tag="slab", name="sa0")
                        sb = slabs.tile([128, BLK], bf16, tag="slab", name="sb0")
                        # zero h rows: gates_0 = W_ih x + b exactly
                        nc.gpsimd.memset(sa[0:64, :], 0.0)
                        nc.gpsimd.memset(sb[64:128, :], 0.0)
                        if do_dma:
                            nc.sync.dma_start(out=sa[64:128, :],
                                              in_=xsl_d[grp["a"] * MAXL])
                            nc.sync.dma_start(out=sb[0:64, :],
                                              in_=xsl_d[grp["b"] * MAXL])
                        st = gstate[g] = {
                            "sa": sa, "sb": sb,
                            "c": statep.tile([128, BLK], bf16, tag="c", name="c"),
                        }
                    else:
                        st = gstate[g]

                    ps = psump.tile([128, 4 * BLK], f32, tag="ps")
                    if do_mm:
                        for q in range(4):
                            qs = slice(64 * q, 64 * q + 64)
                            cs = slice(BLK * q, BLK * q + BLK)
                            nc.tensor.matmul(ps[0:64, cs], ga[:, qs],
                                             st["sa"][:, :],
                                             start=True, stop=True,
                                             tile_position=(0, 0))
                            if b_act:
                                nc.tensor.matmul(ps[64:128, cs], gb[:, qs],
                                                 st["sb"][:, :],
                                                 start=True, stop=True,
                                                 tile_position=(0, 64))

                    # next-step rhs tiles (also the h target / output source)
                    na = slabs.tile([128, BLK], bf16, tag="slab", name="na")
                    if do_dma and t + 1 < La:
                        nc.sync.dma_start(out=na[64:128, :],
                                          in_=xsl_d[grp["a"] * MAXL + t + 1])
                    if b_act:
                        nb_ = slabs.tile([128, BLK], bf16, tag="slab", name="nb")
                        if do_dma and t + 1 < Lb:
                            nc.sync.dma_start(out=nb_[0:64, :],
                                              in_=xsl_d[grp["b"] * MAXL + t + 1])

                    if do_act:
                        sig = sigp.tile([128, 4 * BLK], bf16, tag="sig")
                        nc.scalar.activation(out=sig[sl, :], in_=ps[sl, :],
                                             func=Sigmoid)
                        # t1 = (sg - 0.5) * si  (= i*g / 2)
                        tgt = st["c"] if first else \
                            t1p.tile([128, BLK], bf16, tag="t1")
                        nc.vector.scalar_tensor_tensor(
                            tgt[sl, :], sig[sl, 3 * BLK:4 * BLK], -0.5,
                            sig[sl, 0:BLK], ADD, MULT)
                        if not first:
                            t2 = t2p.tile([128, BLK], bf16, tag="t2")
                            nc.gpsimd.tensor_mul(t2[sl, :],
                                                 sig[sl, BLK:2 * BLK],
                                                 st["c"][sl, :])
                            nc.vector.tensor_add(st["c"][sl, :], tgt[sl, :],
                                                 t2[sl, :])
                        tch = tcp.tile([128, BLK], bf16, tag="tc")
                        nc.scalar.activation(out=tch[sl, :], in_=st["c"][sl, :],
                                             func=Tanh, scale=2.0)
                        nc.vector.tensor_mul(na[0:64, :],
                                             sig[0:64, 2 * BLK:3 * BLK],
                                             tch[0:64, :])
                        if b_act:
                            nc.vector.tensor_mul(nb_[64:128, :],
                                                 sig[64:128, 2 * BLK:3 * BLK],
                                                 tch[64:128, :])

                    if do_act and do_dma:
                        if t == La - 1:
                            nc.sync.dma_start(out=out_d[grp["a"]],
                                              in_=na[0:64, :])
                        if b_act and t == Lb - 1:
                            nc.sync.dma_start(out=out_d[grp["b"]],
                                              in_=nb_[64:128, :])
                        if a["is_ov"] and t in a.get("cap_steps", ()):
                            nc.sync.dma_start(
                                out=ov_d[a["ov_idx"] * MAXL + t],
                                in_=na[0:64, :])
                        if b_act and b["is_ov"] and t in b.get("cap_steps", ()):
                            nc.sync.dma_start(
                                out=ov_d[b["ov_idx"] * MAXL + t],
                                in_=nb_[64:128, :])

                    st["sa"] = na
                    if b_act:
                        st["sb"] = nb_

    nc.compile()
    return nc


# --------------------------------------------------------------------------
# Entry point
# --------------------------------------------------------------------------

def kernel(emb, W_ih, W_hh, b_ih, b_hh, chars, lengths):
    from concourse.bass_utils import run_bass_kernel_spmd

    emb = np.asarray(emb, dtype=np.float32)
    W_ih = np.asarray(W_ih, dtype=np.float32)
    W_hh = np.asarray(W_hh, dtype=np.float32)
    b_ih = np.asarray(b_ih, dtype=np.float32)
    b_hh = np.asarray(b_hh, dtype=np.float32)
    chars = np.asarray(chars)
    lengths_np = np.asarray(lengths)

    n = chars.shape[0]

    # --- weight prep -------------------------------------------------------
    # Banks ordered [i, f, o, g]; g-bank scaled by 2 (tanh via sigmoid).
    scale = np.ones((1, GATE4), dtype=np.float32)
    scale[0, 3 * H:] = 2.0
    WihT = W_ih[_GATE_PERM].T * scale               # [E, 4H]
    WhhT = W_hh[_GATE_PERM].T * scale               # [H, 4H]
    bias = ((b_ih + b_hh)[_GATE_PERM] * scale[0])[None, :]  # [1, 4H]
    gA = np.zeros((128, GATE4), dtype=BF16)
    gA[0:H] = WhhT.astype(BF16)
    gA[H:H + 1] = bias.astype(BF16)
    gA[H + 1:H + 1 + E] = WihT.astype(BF16)
    gB = np.zeros((128, GATE4), dtype=BF16)
    gB[0:E] = WihT.astype(BF16)
    gB[E:E + 1] = bias.astype(BF16)
    gB[64:128] = WhhT.astype(BF16)

    # --- word assignment ---------------------------------------------------
    plan = _plan(lengths_np)
    blocks, groups, sched = plan["blocks"], plan["groups"], plan["sched"]

    sig = (tuple((b["L"], b["is_ov"], b.get("cap_steps", ())) for b in blocks),
           tuple(sched))
    key = hash(sig)
    if key not in _PROGRAM_CACHE:
        _PROGRAM_CACHE[key] = _build_program(sig, blocks, groups, sched,
                                             plan["n_ov"])
    nc = _PROGRAM_CACHE[key]

    emb16 = emb.astype(BF16)
    xsls = _build_xslabs(plan, chars, emb16)
    in_maps = [{"xsl": xsls[c], "ga": gA, "gb": gB} for c in range(NCORES)]

    res = run_bass_kernel_spmd(nc, in_maps, core_ids=list(range(NCORES)))
    kernel._last_nc = nc
    kernel._last_in_maps = in_maps

    # --- gather results ----------------------------------------------------
    outs = np.stack([np.asarray(r["out"], dtype=np.float32)
                     for r in res.results])          # [8, nb, H, BLK]
    ovs = np.stack([np.asarray(r["ov"], dtype=np.float32)
                    for r in res.results])           # [8, n_ov*16, H, BLK]

    result = np.empty((n, H), dtype=np.float32)
    for c in range(NCORES):
        for bi, blk in enumerate(blocks):
            words = plan["assign"][c][bi]
            valid = words >= 0
            if not valid.any():
                continue
            w = words[valid]
            cols = np.nonzero(valid)[0]
            if blk["is_ov"]:
                steps = lengths_np[w].astype(np.int64) - 1
                result[w] = ovs[c, blk["ov_idx"] * MAXL + steps, :, cols]
            else:
                result[w] = outs[c, bi, :, cols]
    return result
